# revision 13
# baseline (speedup 1.0000x reference)
"""Trainium2 Bass kernel for single-head causal attention.

Problem: x[B=4,T=2048,C=1024] -> q,k,v = x@Wq/Wk/Wv [T,64] -> causal softmax(q k^T/sqrt(C)) @ v.

Sharding: 8 cores = 4 batches x 2 interleaved query-tile sets. Core r of a
batch owns global 128-row q-tiles {2m+r : m=0..7} -- interleaving balances
the causal triangle exactly (68 vs 68 blocks) instead of 36 vs 100 for
contiguous halves.

SPMD-uniform trick: each core's x^T copy is column-permuted so its OWN
q-tiles come first (local tiles 0-7), the peer's after (local 8-15). The
causal block structure is then core-independent:
  - local k-tile p<8  (own tiles, global 2p+r): q-window m >= p; the m==p
    block is the diagonal -> multiply by a constant 128x128 triangle.
  - local k-tile p>=8 (peer tiles, global 2(p-8)+1-r): q-window m >= p-8;
    the boundary block m==p-8 is valid only for r==1 -> multiply by a
    per-core all-ones/all-zeros mask. All other blocks are fully valid.
Softmax normalization is fused into the AV matmul by appending a ones
column to V (output row 64 = sum of exp); division happens host-side.

Perf structure:
  - bf16 throughout (fp8 fails the accuracy gate; a pairwise HBM AllGather
    K/V exchange was tried and costs ~35us of collective latency).
  - S^T = k_tile-stationary x q^T-moving; P^T feeds AV with V-natural
    stationary (+ones row). V transposed on the PE (identity transpose).
  - Attention software pipeline: S(p) runs one tile ahead of AV(p-1) so the
    PE never waits on exp/mask latency; boundary masks are in-place
    tensor_muls on the otherwise-idle GPSIMD engine.
  - The other half's projection is emitted in quarter-sized blocks BETWEEN
    attention groups, so exp (ACT) starts ~8us earlier and the PE stream
    stays dense: [own proj | attn 0-3 | proj q2 | attn 8-11 | proj q3 |
    attn 4-7 x 12-15].
  - One shared 4-buffer PSUM pool rotates warm/qk/vtrans/S tiles; 2 banks
    for V projection, 2 for the output accumulator.
"""

import numpy as np
import ml_dtypes

B, T, C, H = 4, 2048, 1024, 64
TQ = 1024          # queries per core
NT = 2048          # kv length per core
NCH = C // 128     # 8 contraction chunks
NKT = NT // 128    # 16 local k-tiles
SCALE = 1.0 / 32.0  # 1/sqrt(C)
VSTRIDE = 80       # bf16 cols per v' tile slot (64 v + 1 ones + pad)
CWA = 1536         # packed weight cols: 1024 wqk + 512 wv
CWB = 320          # packed const cols: 128 tri + 64 idn + 128 gmask

_prog_cache = {}


def _build_program():
    import concourse.mybir as mybir
    from concourse import bacc
    from concourse.tile import TileContext

    fp32 = mybir.dt.float32
    bf16 = mybir.dt.bfloat16
    Exp = mybir.ActivationFunctionType.Exp

    nc = bacc.Bacc("TRN2", target_bir_lowering=False, debug=False)

    xt_d = nc.dram_tensor("xt", [128, 2, NCH, TQ], bf16, kind="ExternalInput")
    cstA_d = nc.dram_tensor("cstA", [128, CWA], bf16, kind="ExternalInput")
    cstB_d = nc.dram_tensor("cstB", [128, CWB], bf16, kind="ExternalInput")
    out_d = nc.dram_tensor("outT", [H + 1, TQ], fp32, kind="ExternalOutput")

    with TileContext(nc) as tc:
        with (
            tc.tile_pool(name="cstp", bufs=1) as cstp,
            tc.tile_pool(name="prj", bufs=1) as prj,
            tc.tile_pool(name="ptp", bufs=6) as ptp,
            tc.tile_pool(name="psX", bufs=4, space="PSUM") as psX,
            tc.tile_pool(name="psB", bufs=2, space="PSUM") as psB,
            tc.tile_pool(name="psO", bufs=1, space="PSUM") as psO,
        ):
            # DMA plan: own half first, split across the two HWDGE queues.
            xt_sb = prj.tile([128, 2, NCH, TQ], bf16, tag="xt")
            cstA_sb = cstp.tile([128, CWA], bf16, tag="cstA")
            cstB_sb = cstp.tile([128, CWB], bf16, tag="cstB")
            nc.scalar.dma_start(out=cstA_sb[:], in_=cstA_d[:])
            nc.sync.dma_start(out=xt_sb[:, 0, 0:4, :], in_=xt_d[:, 0, 0:4, :])
            nc.scalar.dma_start(out=xt_sb[:, 0, 4:8, :], in_=xt_d[:, 0, 4:8, :])
            nc.sync.dma_start(out=xt_sb[:, 1, 0:4, :], in_=xt_d[:, 1, 0:4, :])
            nc.scalar.dma_start(out=cstB_sb[:], in_=cstB_d[:])
            nc.scalar.dma_start(out=xt_sb[:, 1, 4:8, :], in_=xt_d[:, 1, 4:8, :])

            def wqk(c):
                return cstA_sb[:, c * 128:(c + 1) * 128]

            def wv(c):
                return cstA_sb[:, 1024 + c * 64:1024 + (c + 1) * 64]

            tri_sb = cstB_sb[:, 0:128]
            idn_sb = cstB_sb[0:64, 128:192]
            gm_sb = cstB_sb[:, 192:320]

            # persistent tiles
            qT_sb = prj.tile([64, TQ], bf16, tag="qT")
            kT_sb = prj.tile([64, NT], bf16, tag="kT")
            vT_sb = prj.tile([64, NT], bf16, tag="vT")
            vp_sb = prj.tile([128, NKT, VSTRIDE], bf16, tag="vp")
            o_sb = prj.tile([H + 1, TQ], fp32, tag="osb")
            warm_sb = prj.tile([128, 512], bf16, tag="warm")

            nc.vector.memset(vp_sb[:, :, 64:65], 1.0)

            # PE p-state warmup on a local dummy tile (no DMA or memset
            # dependency -- garbage data, discarded result); covers the x^T
            # DMA window to keep the clock ramp alive. Result copied out so
            # the verifier sees a reader.
            warm_ps = psX.tile([128, 512], fp32, tag="x")
            for _ in range(8):
                nc.tensor.matmul(
                    warm_ps[:], warm_sb[:, 0:128], warm_sb[:, 0:512],
                    start=True, stop=True,
                )
            nc.vector.tensor_copy(out=warm_sb[:, 0:64], in_=warm_ps[:, 0:64])
            # warm the Exp table early (1.3us load off the critical path)
            nc.scalar.activation(warm_sb[0:128, 0:1], warm_ps[:, 0:1], Exp, scale=SCALE)

            def emit_own_proj():
                """Project the own half, quarter-pair interleaved per chunk
                so compute tracks DMA arrival."""
                qk_ps = [psX.tile([128, 512], fp32, tag="x", name=f"qk0{i}") for i in range(2)]
                v_ps = [psB.tile([64, 512], fp32, tag="pv", name=f"pv0{i}") for i in range(2)]
                for c in range(NCH):
                    mv = xt_sb[:, 0, c, :]
                    for i in range(2):
                        nc.tensor.matmul(
                            qk_ps[i][:], wqk(c), mv[:, 512 * i:512 * (i + 1)],
                            start=(c == 0), stop=(c == NCH - 1),
                        )
                    for i in range(2):
                        nc.tensor.matmul(
                            v_ps[i][:], wv(c), mv[:, 512 * i:512 * (i + 1)],
                            start=(c == 0), stop=(c == NCH - 1),
                        )
                for i in range(2):
                    sl = slice(512 * i, 512 * (i + 1))
                    nc.vector.tensor_copy(out=qT_sb[:, sl], in_=qk_ps[i][0:64, :])
                    nc.vector.tensor_copy(out=kT_sb[:, sl], in_=qk_ps[i][64:128, :])
                    nc.vector.tensor_copy(out=vT_sb[:, sl], in_=v_ps[i][:])

            def emit_oth_quarter(i):
                """Project other-half cols 512i:512(i+1) as one block
                (slotted between attention groups)."""
                sl = slice(512 * i, 512 * (i + 1))
                osl = slice(1024 + 512 * i, 1024 + 512 * (i + 1))
                qk_ps = psX.tile([128, 512], fp32, tag="x", name=f"qk1{i}")
                for c in range(NCH):
                    nc.tensor.matmul(
                        qk_ps[:], wqk(c), xt_sb[:, 1, c, sl],
                        start=(c == 0), stop=(c == NCH - 1),
                    )
                v_ps = psB.tile([64, 512], fp32, tag="pv", name=f"pv1{i}")
                for c in range(NCH):
                    nc.tensor.matmul(
                        v_ps[:], wv(c), xt_sb[:, 1, c, sl],
                        start=(c == 0), stop=(c == NCH - 1),
                    )
                nc.vector.tensor_copy(out=kT_sb[:, osl], in_=qk_ps[64:128, :])
                nc.vector.tensor_copy(out=vT_sb[:, osl], in_=v_ps[:])

            def emit_vtrans(i):
                """PE-transpose v^T tiles 4i..4i+3 into vp (v-natural)."""
                vt_ps = psX.tile([128, 8, 64], bf16, tag="x", name=f"vt{i}")
                for n in range(4):
                    t = 4 * i + n
                    nc.tensor.transpose(
                        vt_ps[:, n, :], vT_sb[:, 128 * t:128 * (t + 1)], idn_sb
                    )
                for n in range(4):
                    t = 4 * i + n
                    nc.vector.tensor_copy(out=vp_sb[:, t, 0:64], in_=vt_ps[:, n, :])

            o_ps = psO.tile([H + 1, TQ], fp32, tag="o")

            def emit_S(p):
                """S^T matmul + exp per piece; boundary mask in-place on
                GPSIMD (hidden by the one-tile AV lag)."""
                a0 = 128 * (p % 8)
                pieces = [(a0, 512), (512, 1024)] if a0 < 512 else [(a0, 1024)]
                mask = tri_sb if p < 8 else gm_sb
                out = []
                for (lo, hi) in pieces:
                    w = hi - lo
                    s_ps = psX.tile([128, 512], fp32, tag="x", name=f"s{p}_{lo}")
                    nc.tensor.matmul(
                        s_ps[:, 0:w],
                        kT_sb[:, 128 * p:128 * (p + 1)],
                        qT_sb[:, lo:hi],
                        start=True, stop=True,
                    )
                    pt = ptp.tile([128, 512], bf16, tag="pt", name=f"pt{p}_{lo}")
                    nc.scalar.activation(pt[:, 0:w], s_ps[:, 0:w], Exp, scale=SCALE)
                    if lo == a0:
                        nc.gpsimd.tensor_mul(pt[:, 0:128], pt[:, 0:128], mask)
                    out.append((lo, hi, pt))
                return out

            first_av = [True]

            def emit_AV(p, avs):
                for (lo, hi, pt) in avs:
                    nc.tensor.matmul(
                        o_ps[:, lo:hi],
                        vp_sb[:, p, 0:65],
                        pt[:, 0:hi - lo],
                        start=first_av[0], stop=(p == 15 and hi == 1024),
                        skip_group_check=True,
                    )
                first_av[0] = False

            _last_avs = [None]

            def emit_attn(plist, flush=True):
                pend = None
                for p in plist:
                    avs = emit_S(p)
                    if pend is not None:
                        emit_AV(pend[0], pend[1])
                    pend = (p, avs)
                if flush:
                    emit_AV(pend[0], pend[1])
                else:
                    _last_avs[0] = pend[1]

            emit_own_proj()
            emit_vtrans(0)
            emit_vtrans(1)
            emit_attn([0, 1, 2, 3])          # big own windows; ACT starts early
            emit_oth_quarter(0)              # k/v tiles 8-11 (ACT digests attn)
            emit_vtrans(2)
            emit_attn([8, 9, 10, 11])        # big other windows
            # cols 0:512 complete: only p%8<4 windows reach them
            nc.vector.tensor_copy(out=o_sb[:, 0:512], in_=o_ps[:, 0:512])
            nc.sync.dma_start(out=out_d[:, 0:512], in_=o_sb[:, 0:512])
            emit_oth_quarter(1)              # k/v tiles 12-15
            emit_vtrans(3)
            def drain(lo, hi):
                nc.vector.tensor_copy(out=o_sb[:, lo:hi], in_=o_ps[:, lo:hi])
                nc.sync.dma_start(out=out_d[:, lo:hi], in_=o_sb[:, lo:hi])

            emit_attn([4, 12, 5, 13, 6, 14], flush=False)   # small windows
            # cols 512:768 complete after AV(13); [768:896) after AV(14)
            pend7 = emit_S(7)
            drain(512, 768)
            emit_AV(14, _last_avs[0])
            pend15 = emit_S(15)
            emit_AV(7, pend7)
            drain(768, 896)
            emit_AV(15, pend15)
            drain(896, 1024)

    nc.finalize()
    return nc


def _get_program():
    if "nc" not in _prog_cache:
        _prog_cache["nc"] = _build_program()
    return _prog_cache["nc"]


def make_in_maps(x, Wq, Wk, Wv):
    bf16 = ml_dtypes.bfloat16
    wqk = np.concatenate([Wq, Wk], axis=1)          # [C, 128]
    wqk_p = wqk.reshape(8, 128, 128).transpose(1, 0, 2).reshape(128, 1024)
    wv_p = np.asarray(Wv).reshape(8, 128, 64).transpose(1, 0, 2).reshape(128, 512)
    cstA = np.ascontiguousarray(np.concatenate([wqk_p, wv_p], axis=1)).astype(bf16)
    tri = np.triu(np.ones((128, 128), np.float32))  # tri[k,q]=1 iff q>=k
    idn = np.zeros((128, 64), np.float32)
    idn[:64] = np.eye(64, dtype=np.float32)
    in_maps = []
    for core in range(8):
        b, r = core // 2, core % 2
        own = [2 * m + r for m in range(8)]
        other = [2 * m + 1 - r for m in range(8)]
        idx = np.concatenate([np.arange(g * 128, (g + 1) * 128) for g in own + other])
        xp = np.asarray(x[b]).T[:, idx]             # [C, 2048] permuted
        xt = xp.reshape(8, 128, 2, 1024).transpose(1, 2, 0, 3)  # [128,2,8,1024]
        gm = np.full((128, 128), 1.0 if r == 1 else 0.0, np.float32)
        cstB = np.ascontiguousarray(
            np.concatenate([tri, idn, gm], axis=1)
        ).astype(bf16)
        in_maps.append({
            "xt": np.ascontiguousarray(xt).astype(bf16),
            "cstA": cstA,
            "cstB": cstB,
        })
    return in_maps


def postprocess(results):
    out = np.empty((B, T, H), np.float32)
    for core in range(8):
        b, r = core // 2, core % 2
        oT = results[core]["outT"]  # [65, 1024]
        vals = (oT[:H] / oT[H:H + 1]).T.reshape(8, 128, H)
        ob = out[b].reshape(16, 128, H)
        for m in range(8):
            ob[2 * m + r] = vals[m]
    return out


def kernel(x, mask, Wq, Wk, Wv, _trace=False, _tracedir=None):
    from concourse import bass_utils

    nc = _get_program()
    in_maps = make_in_maps(np.asarray(x, np.float32), np.asarray(Wq, np.float32),
                           np.asarray(Wk, np.float32), np.asarray(Wv, np.float32))
    res = bass_utils.run_bass_kernel_spmd(
        nc, in_maps, core_ids=list(range(8)),
        trace=_trace, tmpdir=_tracedir,
    )
    out = postprocess(res.results)
    if _trace:
        return out, res
    return out


# revision 14
# speedup vs baseline: 1.0321x; 1.0321x over previous
"""Trainium2 Bass kernel for single-head causal attention.

Problem: x[B=4,T=2048,C=1024] -> q,k,v = x@Wq/Wk/Wv [T,64] -> causal softmax(q k^T/sqrt(C)) @ v.

Sharding: 8 cores = 4 batches x 2 interleaved query-tile sets. Core r of a
batch owns global 128-row q-tiles {2m+r : m=0..7} -- interleaving balances
the causal triangle exactly (68 vs 68 blocks) instead of 36 vs 100 for
contiguous halves.

SPMD-uniform trick: each core's x^T copy is column-permuted so its OWN
q-tiles come first (local tiles 0-7), the peer's after (local 8-15). The
causal block structure is then core-independent:
  - local k-tile p<8  (own tiles, global 2p+r): q-window m >= p; the m==p
    block is the diagonal -> multiply by a constant 128x128 triangle.
  - local k-tile p>=8 (peer tiles, global 2(p-8)+1-r): q-window m >= p-8;
    the boundary block m==p-8 is valid only for r==1 -> multiply by a
    per-core all-ones/all-zeros mask. All other blocks are fully valid.
Softmax normalization is fused into the AV matmul by appending a ones
column to V (output row 64 = sum of exp); division happens host-side.

Perf structure:
  - bf16 throughout (fp8 fails the accuracy gate; a pairwise HBM AllGather
    K/V exchange was tried and costs ~35us of collective latency).
  - S^T = k_tile-stationary x q^T-moving; P^T feeds AV with V-natural
    stationary (+ones row). V transposed on the PE (identity transpose).
  - Attention software pipeline: S(p) runs one tile ahead of AV(p-1) so the
    PE never waits on exp/mask latency; boundary masks are in-place
    tensor_muls on the otherwise-idle GPSIMD engine.
  - The other half's projection is emitted in quarter-sized blocks BETWEEN
    attention groups, so exp (ACT) starts ~8us earlier and the PE stream
    stays dense: [own proj | attn 0-3 | proj q2 | attn 8-11 | proj q3 |
    attn 4-7 x 12-15].
  - One shared 4-buffer PSUM pool rotates warm/qk/vtrans/S tiles; 2 banks
    for V projection, 2 for the output accumulator.
"""

import numpy as np
import ml_dtypes

B, T, C, H = 4, 2048, 1024, 64
TQ = 1024          # queries per core
NT = 2048          # kv length per core
NCH = C // 128     # 8 contraction chunks
NKT = NT // 128    # 16 local k-tiles
SCALE = 1.0 / 32.0  # 1/sqrt(C)
VSTRIDE = 80       # bf16 cols per v' tile slot (64 v + 1 ones + pad)
CWA = 1536         # packed weight cols: 1024 wqk + 512 wv
CWB = 320          # packed const cols: 128 tri + 64 idn + 128 gmask

_prog_cache = {}


def _build_program():
    import concourse.mybir as mybir
    from concourse import bacc
    from concourse.tile import TileContext

    fp32 = mybir.dt.float32
    bf16 = mybir.dt.bfloat16
    Exp = mybir.ActivationFunctionType.Exp

    nc = bacc.Bacc("TRN2", target_bir_lowering=False, debug=False)

    xt_d = nc.dram_tensor("xt", [128, 2, NCH, TQ], bf16, kind="ExternalInput")
    cstA_d = nc.dram_tensor("cstA", [128, CWA], bf16, kind="ExternalInput")
    cstB_d = nc.dram_tensor("cstB", [128, CWB], bf16, kind="ExternalInput")
    out_d = nc.dram_tensor("outT", [H + 1, TQ], fp32, kind="ExternalOutput")

    with TileContext(nc) as tc:
        with (
            tc.tile_pool(name="cstp", bufs=1) as cstp,
            tc.tile_pool(name="prj", bufs=1) as prj,
            tc.tile_pool(name="ptp", bufs=4) as ptp,
            tc.tile_pool(name="psX", bufs=4, space="PSUM") as psX,
            tc.tile_pool(name="psB", bufs=2, space="PSUM") as psB,
            tc.tile_pool(name="psO", bufs=1, space="PSUM") as psO,
        ):
            # DMA plan: own half first, split across the two HWDGE queues.
            xt_sb = prj.tile([128, 2, NCH, TQ], bf16, tag="xt")
            cstA_sb = cstp.tile([128, CWA], bf16, tag="cstA")
            cstB_sb = cstp.tile([128, CWB], bf16, tag="cstB")
            nc.scalar.dma_start(out=cstA_sb[:], in_=cstA_d[:])
            nc.sync.dma_start(out=xt_sb[:, 0, 0:4, :], in_=xt_d[:, 0, 0:4, :])
            nc.scalar.dma_start(out=xt_sb[:, 0, 4:8, :], in_=xt_d[:, 0, 4:8, :])
            nc.sync.dma_start(out=xt_sb[:, 1, 0:4, :], in_=xt_d[:, 1, 0:4, :])
            nc.scalar.dma_start(out=cstB_sb[:], in_=cstB_d[:])
            nc.scalar.dma_start(out=xt_sb[:, 1, 4:8, :], in_=xt_d[:, 1, 4:8, :])

            def wqk(c):
                return cstA_sb[:, c * 128:(c + 1) * 128]

            def wv(c):
                return cstA_sb[:, 1024 + c * 64:1024 + (c + 1) * 64]

            tri_sb = cstB_sb[:, 0:128]
            idn_sb = cstB_sb[0:64, 128:192]
            gm_sb = cstB_sb[:, 192:320]

            # persistent tiles
            qT_sb = prj.tile([64, TQ], bf16, tag="qT")
            kT_sb = prj.tile([64, NT], bf16, tag="kT")
            vT_sb = prj.tile([64, NT], bf16, tag="vT")
            vp_sb = prj.tile([128, NKT, VSTRIDE], bf16, tag="vp")
            o_sb = prj.tile([H + 1, TQ], fp32, tag="osb")
            warm_sb = prj.tile([128, 512], bf16, tag="warm")

            nc.vector.memset(vp_sb[:, :, 64:65], 1.0)
            nc.vector.memset(warm_sb[:, 0:128], 0.0)

            # PE p-state warmup on a local dummy tile (no DMA dependency);
            # result copied out so the verifier sees a reader.
            warm_ps = psX.tile([128, 512], fp32, tag="x")
            for _ in range(8):
                nc.tensor.matmul(
                    warm_ps[:], warm_sb[:, 0:128], warm_sb[:, 0:512],
                    start=True, stop=True,
                )
            nc.vector.tensor_copy(out=warm_sb[:, 0:64], in_=warm_ps[:, 0:64])
            # warm the Exp table early (1.3us load off the critical path)
            nc.scalar.activation(warm_sb[0:128, 0:1], warm_ps[:, 0:1], Exp, scale=SCALE)

            def emit_own_proj():
                """Project the own half, quarter-pair interleaved per chunk
                so compute tracks DMA arrival."""
                qk_ps = [psX.tile([128, 512], fp32, tag="x", name=f"qk0{i}") for i in range(2)]
                v_ps = [psB.tile([64, 512], fp32, tag="pv", name=f"pv0{i}") for i in range(2)]
                for c in range(NCH):
                    mv = xt_sb[:, 0, c, :]
                    for i in range(2):
                        nc.tensor.matmul(
                            qk_ps[i][:], wqk(c), mv[:, 512 * i:512 * (i + 1)],
                            start=(c == 0), stop=(c == NCH - 1),
                        )
                    for i in range(2):
                        nc.tensor.matmul(
                            v_ps[i][:], wv(c), mv[:, 512 * i:512 * (i + 1)],
                            start=(c == 0), stop=(c == NCH - 1),
                        )
                for i in range(2):
                    sl = slice(512 * i, 512 * (i + 1))
                    nc.vector.tensor_copy(out=qT_sb[:, sl], in_=qk_ps[i][0:64, :])
                    nc.vector.tensor_copy(out=kT_sb[:, sl], in_=qk_ps[i][64:128, :])
                    nc.vector.tensor_copy(out=vT_sb[:, sl], in_=v_ps[i][:])

            def emit_oth_quarter(i):
                """Project other-half cols 512i:512(i+1) as one block
                (slotted between attention groups)."""
                sl = slice(512 * i, 512 * (i + 1))
                osl = slice(1024 + 512 * i, 1024 + 512 * (i + 1))
                qk_ps = psX.tile([128, 512], fp32, tag="x", name=f"qk1{i}")
                for c in range(NCH):
                    nc.tensor.matmul(
                        qk_ps[:], wqk(c), xt_sb[:, 1, c, sl],
                        start=(c == 0), stop=(c == NCH - 1),
                    )
                v_ps = psB.tile([64, 512], fp32, tag="pv", name=f"pv1{i}")
                for c in range(NCH):
                    nc.tensor.matmul(
                        v_ps[:], wv(c), xt_sb[:, 1, c, sl],
                        start=(c == 0), stop=(c == NCH - 1),
                    )
                nc.vector.tensor_copy(out=kT_sb[:, osl], in_=qk_ps[64:128, :])
                nc.vector.tensor_copy(out=vT_sb[:, osl], in_=v_ps[:])

            def emit_vtrans(i):
                """PE-transpose v^T tiles 4i..4i+3 into vp (v-natural)."""
                vt_ps = psX.tile([128, 8, 64], bf16, tag="x", name=f"vt{i}")
                for n in range(4):
                    t = 4 * i + n
                    nc.tensor.transpose(
                        vt_ps[:, n, :], vT_sb[:, 128 * t:128 * (t + 1)], idn_sb
                    )
                for n in range(4):
                    t = 4 * i + n
                    nc.vector.tensor_copy(out=vp_sb[:, t, 0:64], in_=vt_ps[:, n, :])

            o_ps = psO.tile([H + 1, TQ], fp32, tag="o")

            def emit_S(p):
                """S^T matmul + exp per piece; boundary mask in-place on
                GPSIMD (hidden by the one-tile AV lag)."""
                a0 = 128 * (p % 8)
                pieces = [(a0, 512), (512, 1024)] if a0 < 512 else [(a0, 1024)]
                mask = tri_sb if p < 8 else gm_sb
                out = []
                for (lo, hi) in pieces:
                    w = hi - lo
                    s_ps = psX.tile([128, 512], fp32, tag="x", name=f"s{p}_{lo}")
                    nc.tensor.matmul(
                        s_ps[:, 0:w],
                        kT_sb[:, 128 * p:128 * (p + 1)],
                        qT_sb[:, lo:hi],
                        start=True, stop=True,
                    )
                    pt = ptp.tile([128, 512], bf16, tag="pt", name=f"pt{p}_{lo}")
                    nc.scalar.activation(pt[:, 0:w], s_ps[:, 0:w], Exp, scale=SCALE)
                    if lo == a0:
                        nc.gpsimd.tensor_mul(pt[:, 0:128], pt[:, 0:128], mask)
                    out.append((lo, hi, pt))
                return out

            first_av = [True]

            def emit_AV(p, avs):
                for (lo, hi, pt) in avs:
                    nc.tensor.matmul(
                        o_ps[:, lo:hi],
                        vp_sb[:, p, 0:65],
                        pt[:, 0:hi - lo],
                        start=first_av[0], stop=(p == 15 and hi == 1024),
                        skip_group_check=True,
                    )
                first_av[0] = False

            _last_avs = [None]

            def emit_attn(plist, flush=True):
                pend = None
                for p in plist:
                    avs = emit_S(p)
                    if pend is not None:
                        emit_AV(pend[0], pend[1])
                    pend = (p, avs)
                if flush:
                    emit_AV(pend[0], pend[1])
                else:
                    _last_avs[0] = pend[1]

            emit_own_proj()
            emit_vtrans(0)
            emit_vtrans(1)
            emit_attn([0, 1, 2, 3])          # big own windows; ACT starts early
            emit_oth_quarter(0)              # k/v tiles 8-11 (ACT digests attn)
            emit_vtrans(2)
            emit_attn([8, 9, 10, 11])        # big other windows
            # cols 0:512 complete: only p%8<4 windows reach them
            nc.vector.tensor_copy(out=o_sb[:, 0:512], in_=o_ps[:, 0:512])
            nc.sync.dma_start(out=out_d[:, 0:512], in_=o_sb[:, 0:512])
            emit_oth_quarter(1)              # k/v tiles 12-15
            emit_vtrans(3)
            emit_attn([4, 12, 5, 13, 6, 14, 7, 15])   # small windows
            nc.vector.tensor_copy(out=o_sb[:, 512:1024], in_=o_ps[:, 512:1024])
            nc.sync.dma_start(out=out_d[:, 512:1024], in_=o_sb[:, 512:1024])

    nc.finalize()
    return nc


def _get_program():
    if "nc" not in _prog_cache:
        _prog_cache["nc"] = _build_program()
    return _prog_cache["nc"]


def make_in_maps(x, Wq, Wk, Wv):
    bf16 = ml_dtypes.bfloat16
    wqk = np.concatenate([Wq, Wk], axis=1)          # [C, 128]
    wqk_p = wqk.reshape(8, 128, 128).transpose(1, 0, 2).reshape(128, 1024)
    wv_p = np.asarray(Wv).reshape(8, 128, 64).transpose(1, 0, 2).reshape(128, 512)
    cstA = np.ascontiguousarray(np.concatenate([wqk_p, wv_p], axis=1)).astype(bf16)
    tri = np.triu(np.ones((128, 128), np.float32))  # tri[k,q]=1 iff q>=k
    idn = np.zeros((128, 64), np.float32)
    idn[:64] = np.eye(64, dtype=np.float32)
    in_maps = []
    for core in range(8):
        b, r = core // 2, core % 2
        own = [2 * m + r for m in range(8)]
        other = [2 * m + 1 - r for m in range(8)]
        idx = np.concatenate([np.arange(g * 128, (g + 1) * 128) for g in own + other])
        xp = np.asarray(x[b]).T[:, idx]             # [C, 2048] permuted
        xt = xp.reshape(8, 128, 2, 1024).transpose(1, 2, 0, 3)  # [128,2,8,1024]
        gm = np.full((128, 128), 1.0 if r == 1 else 0.0, np.float32)
        cstB = np.ascontiguousarray(
            np.concatenate([tri, idn, gm], axis=1)
        ).astype(bf16)
        in_maps.append({
            "xt": np.ascontiguousarray(xt).astype(bf16),
            "cstA": cstA,
            "cstB": cstB,
        })
    return in_maps


def postprocess(results):
    out = np.empty((B, T, H), np.float32)
    for core in range(8):
        b, r = core // 2, core % 2
        oT = results[core]["outT"]  # [65, 1024]
        vals = (oT[:H] / oT[H:H + 1]).T.reshape(8, 128, H)
        ob = out[b].reshape(16, 128, H)
        for m in range(8):
            ob[2 * m + r] = vals[m]
    return out


def kernel(x, mask, Wq, Wk, Wv, _trace=False, _tracedir=None):
    from concourse import bass_utils

    nc = _get_program()
    in_maps = make_in_maps(np.asarray(x, np.float32), np.asarray(Wq, np.float32),
                           np.asarray(Wk, np.float32), np.asarray(Wv, np.float32))
    res = bass_utils.run_bass_kernel_spmd(
        nc, in_maps, core_ids=list(range(8)),
        trace=_trace, tmpdir=_tracedir,
    )
    out = postprocess(res.results)
    if _trace:
        return out, res
    return out
